# revision 5
# baseline (speedup 1.0000x reference)
"""Trainium2 Bass kernel for nn_NetStackedHourglass_2 keypoint reduction.

Full inputs in, full output out.  Pure data-parallel across 8 NeuronCores
(32 batches each).  Host pre-casts the five input streams to bf16 and
interleaves them into ONE DRAM tensor per core laid out
[ROWS, NCHUNK, 7, CHUNK] with planes (m, fd, bd, fvx, fvy, bvx, bvy): the
kernel is HBM-bandwidth bound, so bf16 halves device traffic (quantization
error ~2.5e-4 rel, ~80x inside the 2e-2 gate) and doubles DVE throughput
via the 2x packed mode.  Each (group, half) needs a single 3.67 MB DMA
(28 KB contiguous per partition), alternated across both HWDGE rings.
All compute slices are contiguous; DVE ops write in place over dead
planes; accumulators stay f32; msum runs on ScalarE with an SBUF scratch
output.  The tiny [B,20,*] -> [B,21,2] keypoint assembly runs on host.
"""

import sys

if "/opt/trn_rl_repo" not in sys.path:
    sys.path.insert(0, "/opt/trn_rl_repo")

import numpy as np

import concourse.bass as bass
import concourse.tile as tile
from concourse import mybir
from concourse.bass_utils import run_bass_kernel_spmd

N_CORES = 8
B_FULL = 256
B_SHARD = B_FULL // N_CORES  # 32
C = 20
RES = 64
SPATIAL = RES * RES          # 4096
ROWS = B_SHARD * C           # 640 (b,c) rows per core
P = 128                      # partitions
GROUPS = ROWS // P           # 5
CHUNK = 4096                 # spatial elements per tile (full spatial row)
NCHUNK = SPATIAL // CHUNK    # 1
EPS = 1e-6

F32 = mybir.dt.float32
BF16 = mybir.dt.bfloat16


def _build_program(repeat: int = 1) -> bass.Bass:
    nc = bass.Bass()

    comb = nc.declare_dram_parameter(
        "comb", [ROWS, NCHUNK, 7, CHUNK], BF16, isOutput=False
    )
    # rows: locx for chunk 0, then locy (locx for chunk ci = locx0 + 32*ci,
    # fixed up on host via msum)
    loc = nc.declare_dram_parameter("loc_const", [2, CHUNK], BF16, isOutput=False)
    # raw per-chunk accumulators: 0:fx 1:fy 2:bx 3:by 4:lx 5:ly 6:msum 7:pad
    stats = nc.declare_dram_parameter("stats", [ROWS, NCHUNK * 8], F32, isOutput=True)

    MULT = mybir.AluOpType.mult

    with tile.TileContext(nc) as tc:
        with (
            tc.tile_pool(name="singles", bufs=1) as singles,
            tc.tile_pool(name="io", bufs=2) as io,
            tc.tile_pool(name="scr", bufs=2) as scr,
            tc.tile_pool(name="acc", bufs=4) as accp,
        ):
            locx = singles.tile([P, CHUNK], BF16, tag="locx")
            nc.gpsimd.dma_start(out=locx, in_=loc[0:1, :].to_broadcast([P, CHUNK]))
            locy = singles.tile([P, CHUNK], BF16, tag="locy")
            nc.gpsimd.dma_start(out=locy, in_=loc[1:2, :].to_broadcast([P, CHUNK]))

            rings = [nc.sync, nc.scalar]
            i = 0
            for g in range(GROUPS * repeat):
                g = g % GROUPS
                r0 = g * P
                acc_t = accp.tile([P, NCHUNK, 8], F32, tag="acc")
                nc.vector.memset(acc_t[:, :, 7:8], 0.0)
                for ci in range(NCHUNK):
                    t = io.tile([P, 7, CHUNK], BF16, tag="comb")
                    rings[i % 2].dma_start(out=t, in_=comb[r0 : r0 + P, ci, :, :])
                    i += 1

                    # msum on ScalarE (activation w/ accumulate; out -> SBUF
                    # scratch)
                    mscr = scr.tile([P, CHUNK], BF16, tag="mscr")
                    nc.scalar.activation(
                        out=mscr,
                        in_=t[:, 0],
                        func=mybir.ActivationFunctionType.Copy,
                        accum_out=acc_t[:, ci, 6:7],
                    )

                    # t_f = m*fd over fd's plane; t_b = m*bd over bd's plane
                    nc.vector.tensor_mul(t[:, 1], t[:, 0], t[:, 1])
                    nc.vector.tensor_mul(t[:, 2], t[:, 0], t[:, 2])

                    def fused_dot(q_in, q_t, qi, scale):
                        # t[:,q_in] = (t[:,q_in] * scale) * q_t ; accum = sum
                        nc.vector.scalar_tensor_tensor(
                            out=t[:, q_in],
                            in0=t[:, q_in],
                            scalar=scale,
                            in1=q_t,
                            op0=MULT,
                            op1=MULT,
                            accum_out=acc_t[:, ci, qi : qi + 1],
                        )

                    fused_dot(3, t[:, 1], 0, 64.0)   # fvx * t_f
                    fused_dot(4, t[:, 1], 1, 64.0)   # fvy * t_f (t_f dead)
                    fused_dot(5, t[:, 2], 2, 64.0)   # bvx * t_b
                    # bvy vote: plane 6 is host-prescaled by 64, so this is a
                    # packed TT product; its sum joins the ScalarE passes
                    nc.vector.tensor_mul(t[:, 6], t[:, 2], t[:, 6])
                    # mask-location moments: packed TT products over dead
                    # planes 1/2, sums on ScalarE (which has slack)
                    nc.vector.tensor_mul(t[:, 1], t[:, 0], locx)
                    nc.vector.tensor_mul(t[:, 2], t[:, 0], locy)
                    for qi, plane in ((4, 1), (5, 2), (3, 6)):
                        lscr = scr.tile([P, CHUNK], BF16, tag="lscr")
                        nc.scalar.activation(
                            out=lscr,
                            in_=t[:, plane],
                            func=mybir.ActivationFunctionType.Copy,
                            accum_out=acc_t[:, ci, qi : qi + 1],
                        )

                nc.sync.dma_start(
                    out=stats[r0 : r0 + P, :],
                    in_=acc_t.rearrange("p a b -> p (a b)"),
                )

    from concourse.library_overlay import lower_extended_insts

    lower_extended_insts(nc)
    _legalize_waits(nc)
    return nc


def _legalize_waits(nc) -> None:
    """walrus codegen allows 1 sync-wait per instruction (2 for
    EventSemaphore). Hoist excess waits onto EventSemaphore carriers
    inserted just before the offending instruction on the same engine."""
    for f in nc.m.functions:
        for blk in f.blocks:
            insts = blk.instructions
            new_list = []
            changed = False
            for ins in insts:
                si = getattr(ins, "sync_info", None)
                ow = list(si.on_wait) if (si is not None and si.on_wait) else []
                cap = 2 if isinstance(ins, mybir.InstEventSemaphore) else 1
                if len(ow) > cap:
                    excess, keep = ow[:-cap], ow[-cap:]
                    for j in range(0, len(excess), 2):
                        ev = mybir.InstEventSemaphore(
                            name=f"{ins.name}-lw{j}", ins=[], outs=[]
                        )
                        ev.engine = ins.engine
                        ev.sync_info = mybir.SyncInfo(
                            on_wait=excess[j : j + 2], on_update=[]
                        )
                        new_list.append(ev)
                    ins.sync_info = mybir.SyncInfo(
                        on_wait=keep,
                        on_update=list(si.on_update) if si.on_update else [],
                    )
                    changed = True
                new_list.append(ins)
            if changed:
                blk.instructions.clear()
                blk.instructions.extend(new_list)


_PROGRAM_CACHE: dict = {}


def _get_program() -> bass.Bass:
    if "nc" not in _PROGRAM_CACHE:
        _PROGRAM_CACHE["nc"] = _build_program()
    return _PROGRAM_CACHE["nc"]


def _run_device(in_maps, trace=False, **kwargs):
    nc = _get_program()
    return run_bass_kernel_spmd(nc, in_maps, list(range(N_CORES)), trace=trace, **kwargs)


def _make_in_maps(front_vec, front_dis, back_vec, back_dis, ske_mask):
    import ml_dtypes

    bf = ml_dtypes.bfloat16
    fv = np.asarray(front_vec, dtype=np.float32)
    fd = np.asarray(front_dis, dtype=np.float32)
    bv = np.asarray(back_vec, dtype=np.float32)
    bd = np.asarray(back_dis, dtype=np.float32)
    m = np.asarray(ske_mask, dtype=np.float32)
    B = m.shape[0]
    R = B * C  # 5120 rows total

    comb = np.empty((R, NCHUNK, 7, CHUNK), bf)

    def rows(x):  # [R, SPATIAL] -> [R, NCHUNK, CHUNK]
        return x.reshape(R, NCHUNK, CHUNK)

    comb[:, :, 0] = rows(m.reshape(R, SPATIAL)).astype(bf)
    comb[:, :, 1] = rows(fd.reshape(R, SPATIAL)).astype(bf)
    comb[:, :, 2] = rows(bd.reshape(R, SPATIAL)).astype(bf)
    fvp = fv.reshape(R, SPATIAL, 2)
    bvp = bv.reshape(R, SPATIAL, 2)
    comb[:, :, 3] = fvp[:, :, 0].reshape(R, NCHUNK, CHUNK).astype(bf)
    comb[:, :, 4] = fvp[:, :, 1].reshape(R, NCHUNK, CHUNK).astype(bf)
    comb[:, :, 5] = bvp[:, :, 0].reshape(R, NCHUNK, CHUNK).astype(bf)
    comb[:, :, 6] = (bvp[:, :, 1] * np.float32(RES)).reshape(R, NCHUNK, CHUNK).astype(bf)

    p = np.arange(CHUNK)
    loc_const = np.ascontiguousarray(
        np.stack([(p // RES).astype(bf), (p % RES).astype(bf)])
    )

    in_maps = []
    for i in range(N_CORES):
        sl = slice(i * ROWS, (i + 1) * ROWS)
        in_maps.append({"comb": comb[sl], "loc_const": loc_const})
    return in_maps


def _assemble(stats: np.ndarray) -> np.ndarray:
    """stats: [B, 20, NCHUNK*8] raw accumulators -> kp [B, 21, 2]."""
    B = stats.shape[0]
    acc = stats.reshape(B, C, NCHUNK, 8).astype(np.float32)
    s = acc.sum(axis=2)
    for ci in range(1, NCHUNK):
        s[:, :, 4] += np.float32(ci * (CHUNK // RES)) * acc[:, :, ci, 6]
    msum = s[:, :, 6]
    r = np.float32(1.0) / (msum + np.float32(EPS))
    F_ = np.stack([(s[:, :, 0] + s[:, :, 4]) * r, (s[:, :, 1] + s[:, :, 5]) * r], -1)
    Bk = np.stack([(s[:, :, 2] + s[:, :, 4]) * r, (s[:, :, 3] + s[:, :, 5]) * r], -1)

    root_terms = np.where(
        (msum[:, ::4] != 0.0)[..., None], Bk[:, ::4], np.float32(0.0)
    )  # [B,5,2]
    kp0 = root_terms.sum(axis=1, dtype=np.float32) / np.float32(5.0)  # [B,2]

    Fg = F_.reshape(B, 5, 4, 2)
    Bg = Bk.reshape(B, 5, 4, 2)
    tail = np.stack(
        [
            Fg[:, :, 3],
            (Fg[:, :, 2] + Bg[:, :, 3]) * np.float32(0.5),
            (Fg[:, :, 1] + Bg[:, :, 2]) * np.float32(0.5),
            (Fg[:, :, 0] + Bg[:, :, 1]) * np.float32(0.5),
        ],
        axis=2,
    )  # [B,5,4,2]
    kp = np.concatenate([kp0[:, None], tail.reshape(B, 20, 2)], axis=1)
    return (kp * np.float32(4.0)).astype(np.float32)


def kernel(front_vec, front_dis, back_vec, back_dis, ske_mask) -> np.ndarray:
    in_maps = _make_in_maps(front_vec, front_dis, back_vec, back_dis, ske_mask)
    res = _run_device(in_maps)
    stats = np.stack([np.asarray(res.results[i]["stats"]) for i in range(N_CORES)])
    stats = stats.reshape(B_FULL, C, NCHUNK * 8)
    return _assemble(stats)


# revision 6
# speedup vs baseline: 1.2850x; 1.2850x over previous
"""Trainium2 Bass kernel for nn_NetStackedHourglass_2 keypoint reduction, v12.

Full inputs in, full output out.  Pure data-parallel across 8 NeuronCores
(32 batches each).  Host pre-casts the five input streams to bf16 and
interleaves them into ONE DRAM tensor per core laid out
[ROWS, NCHUNK, 7, CHUNK] with planes (m, fd, bd, fvx, fvy, bvx, bvy): the
kernel is HBM-bandwidth bound, so bf16 halves device traffic (quantization
error ~2.5e-4 rel, ~80x inside the 2e-2 gate) and doubles DVE throughput
via the 2x packed mode.  Each (group, half) needs a single 3.67 MB DMA
(28 KB contiguous per partition), alternated across both HWDGE rings.
All compute slices are contiguous; DVE ops write in place over dead
planes; accumulators stay f32; msum runs on ScalarE with an SBUF scratch
output.  The tiny [B,20,*] -> [B,21,2] keypoint assembly runs on host.
"""

import sys

if "/opt/trn_rl_repo" not in sys.path:
    sys.path.insert(0, "/opt/trn_rl_repo")

import numpy as np

import concourse.bass as bass
import concourse.tile as tile
from concourse import mybir
from concourse.bass_utils import run_bass_kernel_spmd

N_CORES = 8
B_FULL = 256
B_SHARD = B_FULL // N_CORES  # 32
C = 20
RES = 64
SPATIAL = RES * RES          # 4096
ROWS = B_SHARD * C           # 640 (b,c) rows per core
P = 128                      # partitions
GROUPS = ROWS // P           # 5
CHUNK = 4096                 # spatial elements per tile (full spatial row)
NCHUNK = SPATIAL // CHUNK    # 1
EPS = 1e-6

F32 = mybir.dt.float32
BF16 = mybir.dt.bfloat16


def _build_program(repeat: int = 1) -> bass.Bass:
    nc = bass.Bass()

    comb = nc.declare_dram_parameter(
        "comb", [ROWS, NCHUNK, 7, CHUNK], BF16, isOutput=False
    )
    # rows: locx for chunk 0, then locy (locx for chunk ci = locx0 + 32*ci,
    # fixed up on host via msum)
    loc = nc.declare_dram_parameter("loc_const", [2, CHUNK], BF16, isOutput=False)
    # raw per-chunk accumulators: 0:fx 1:fy 2:bx 3:by 4:lx 5:ly 6:msum 7:pad
    stats = nc.declare_dram_parameter("stats", [ROWS, NCHUNK * 8], F32, isOutput=True)

    MULT = mybir.AluOpType.mult

    with tile.TileContext(nc) as tc:
        with (
            tc.tile_pool(name="singles", bufs=1) as singles,
            tc.tile_pool(name="io", bufs=3) as io,
            tc.tile_pool(name="scr", bufs=2) as scr,
            tc.tile_pool(name="acc", bufs=4) as accp,
        ):
            locx = singles.tile([P, CHUNK], BF16, tag="locx")
            nc.gpsimd.dma_start(out=locx, in_=loc[0:1, :].to_broadcast([P, CHUNK]))
            locy = singles.tile([P, CHUNK], BF16, tag="locy")
            nc.gpsimd.dma_start(out=locy, in_=loc[1:2, :].to_broadcast([P, CHUNK]))

            rings = [nc.sync, nc.scalar]
            i = 0
            for g in range(GROUPS * repeat):
                g = g % GROUPS
                r0 = g * P
                acc_t = accp.tile([P, NCHUNK, 8], F32, tag="acc")
                nc.vector.memset(acc_t[:, :, 7:8], 0.0)
                for ci in range(NCHUNK):
                    t = io.tile([P, 7, CHUNK], BF16, tag="comb")
                    rings[i % 2].dma_start(out=t, in_=comb[r0 : r0 + P, ci, :, :])
                    i += 1

                    # msum on ScalarE (activation w/ accumulate; out -> SBUF
                    # scratch)
                    mscr = scr.tile([P, CHUNK], BF16, tag="mscr")
                    nc.scalar.activation(
                        out=mscr,
                        in_=t[:, 0],
                        func=mybir.ActivationFunctionType.Copy,
                        accum_out=acc_t[:, ci, 6:7],
                    )

                    # t_f = m*fd over fd's plane; t_b = m*bd over bd's plane
                    nc.vector.tensor_mul(t[:, 1], t[:, 0], t[:, 1])
                    nc.vector.tensor_mul(t[:, 2], t[:, 0], t[:, 2])

                    def fused_dot(q_in, q_t, qi, scale):
                        # t[:,q_in] = (t[:,q_in] * scale) * q_t ; accum = sum
                        nc.vector.scalar_tensor_tensor(
                            out=t[:, q_in],
                            in0=t[:, q_in],
                            scalar=scale,
                            in1=q_t,
                            op0=MULT,
                            op1=MULT,
                            accum_out=acc_t[:, ci, qi : qi + 1],
                        )

                    fused_dot(3, t[:, 1], 0, 64.0)   # fvx * t_f
                    fused_dot(4, t[:, 1], 1, 64.0)   # fvy * t_f (t_f dead)
                    fused_dot(5, t[:, 2], 2, 64.0)   # bvx * t_b
                    # bvy vote: plane 6 is host-prescaled by 64, so this is a
                    # packed TT product; its sum joins the ScalarE passes
                    nc.vector.tensor_mul(t[:, 6], t[:, 2], t[:, 6])
                    # mask-location moments: packed TT products over dead
                    # planes 1/2, sums on ScalarE (which has slack)
                    nc.vector.tensor_mul(t[:, 1], t[:, 0], locx)
                    nc.vector.tensor_mul(t[:, 2], t[:, 0], locy)
                    for qi, plane in ((4, 1), (5, 2), (3, 6)):
                        lscr = scr.tile([P, CHUNK], BF16, tag="mscr")
                        nc.scalar.activation(
                            out=lscr,
                            in_=t[:, plane],
                            func=mybir.ActivationFunctionType.Copy,
                            accum_out=acc_t[:, ci, qi : qi + 1],
                        )

                nc.sync.dma_start(
                    out=stats[r0 : r0 + P, :],
                    in_=acc_t.rearrange("p a b -> p (a b)"),
                )

    from concourse.library_overlay import lower_extended_insts

    lower_extended_insts(nc)
    _legalize_waits(nc)
    return nc


def _legalize_waits(nc) -> None:
    """walrus codegen allows 1 sync-wait per instruction (2 for
    EventSemaphore). Hoist excess waits onto EventSemaphore carriers
    inserted just before the offending instruction on the same engine."""
    for f in nc.m.functions:
        for blk in f.blocks:
            insts = blk.instructions
            new_list = []
            changed = False
            for ins in insts:
                si = getattr(ins, "sync_info", None)
                ow = list(si.on_wait) if (si is not None and si.on_wait) else []
                cap = 2 if isinstance(ins, mybir.InstEventSemaphore) else 1
                if len(ow) > cap:
                    excess, keep = ow[:-cap], ow[-cap:]
                    for j in range(0, len(excess), 2):
                        ev = mybir.InstEventSemaphore(
                            name=f"{ins.name}-lw{j}", ins=[], outs=[]
                        )
                        ev.engine = ins.engine
                        ev.sync_info = mybir.SyncInfo(
                            on_wait=excess[j : j + 2], on_update=[]
                        )
                        new_list.append(ev)
                    ins.sync_info = mybir.SyncInfo(
                        on_wait=keep,
                        on_update=list(si.on_update) if si.on_update else [],
                    )
                    changed = True
                new_list.append(ins)
            if changed:
                blk.instructions.clear()
                blk.instructions.extend(new_list)


_PROGRAM_CACHE: dict = {}


def _get_program() -> bass.Bass:
    if "nc" not in _PROGRAM_CACHE:
        _PROGRAM_CACHE["nc"] = _build_program()
    return _PROGRAM_CACHE["nc"]


def _run_device(in_maps, trace=False, **kwargs):
    nc = _get_program()
    return run_bass_kernel_spmd(nc, in_maps, list(range(N_CORES)), trace=trace, **kwargs)


def _make_in_maps(front_vec, front_dis, back_vec, back_dis, ske_mask):
    import ml_dtypes

    bf = ml_dtypes.bfloat16
    fv = np.asarray(front_vec, dtype=np.float32)
    fd = np.asarray(front_dis, dtype=np.float32)
    bv = np.asarray(back_vec, dtype=np.float32)
    bd = np.asarray(back_dis, dtype=np.float32)
    m = np.asarray(ske_mask, dtype=np.float32)
    B = m.shape[0]
    R = B * C  # 5120 rows total

    comb = np.empty((R, NCHUNK, 7, CHUNK), bf)

    def rows(x):  # [R, SPATIAL] -> [R, NCHUNK, CHUNK]
        return x.reshape(R, NCHUNK, CHUNK)

    comb[:, :, 0] = rows(m.reshape(R, SPATIAL)).astype(bf)
    comb[:, :, 1] = rows(fd.reshape(R, SPATIAL)).astype(bf)
    comb[:, :, 2] = rows(bd.reshape(R, SPATIAL)).astype(bf)
    fvp = fv.reshape(R, SPATIAL, 2)
    bvp = bv.reshape(R, SPATIAL, 2)
    comb[:, :, 3] = fvp[:, :, 0].reshape(R, NCHUNK, CHUNK).astype(bf)
    comb[:, :, 4] = fvp[:, :, 1].reshape(R, NCHUNK, CHUNK).astype(bf)
    comb[:, :, 5] = bvp[:, :, 0].reshape(R, NCHUNK, CHUNK).astype(bf)
    comb[:, :, 6] = (bvp[:, :, 1] * np.float32(RES)).reshape(R, NCHUNK, CHUNK).astype(bf)

    p = np.arange(CHUNK)
    loc_const = np.ascontiguousarray(
        np.stack([(p // RES).astype(bf), (p % RES).astype(bf)])
    )

    in_maps = []
    for i in range(N_CORES):
        sl = slice(i * ROWS, (i + 1) * ROWS)
        in_maps.append({"comb": comb[sl], "loc_const": loc_const})
    return in_maps


def _assemble(stats: np.ndarray) -> np.ndarray:
    """stats: [B, 20, NCHUNK*8] raw accumulators -> kp [B, 21, 2]."""
    B = stats.shape[0]
    acc = stats.reshape(B, C, NCHUNK, 8).astype(np.float32)
    s = acc.sum(axis=2)
    for ci in range(1, NCHUNK):
        s[:, :, 4] += np.float32(ci * (CHUNK // RES)) * acc[:, :, ci, 6]
    msum = s[:, :, 6]
    r = np.float32(1.0) / (msum + np.float32(EPS))
    F_ = np.stack([(s[:, :, 0] + s[:, :, 4]) * r, (s[:, :, 1] + s[:, :, 5]) * r], -1)
    Bk = np.stack([(s[:, :, 2] + s[:, :, 4]) * r, (s[:, :, 3] + s[:, :, 5]) * r], -1)

    root_terms = np.where(
        (msum[:, ::4] != 0.0)[..., None], Bk[:, ::4], np.float32(0.0)
    )  # [B,5,2]
    kp0 = root_terms.sum(axis=1, dtype=np.float32) / np.float32(5.0)  # [B,2]

    Fg = F_.reshape(B, 5, 4, 2)
    Bg = Bk.reshape(B, 5, 4, 2)
    tail = np.stack(
        [
            Fg[:, :, 3],
            (Fg[:, :, 2] + Bg[:, :, 3]) * np.float32(0.5),
            (Fg[:, :, 1] + Bg[:, :, 2]) * np.float32(0.5),
            (Fg[:, :, 0] + Bg[:, :, 1]) * np.float32(0.5),
        ],
        axis=2,
    )  # [B,5,4,2]
    kp = np.concatenate([kp0[:, None], tail.reshape(B, 20, 2)], axis=1)
    return (kp * np.float32(4.0)).astype(np.float32)


def kernel(front_vec, front_dis, back_vec, back_dis, ske_mask) -> np.ndarray:
    in_maps = _make_in_maps(front_vec, front_dis, back_vec, back_dis, ske_mask)
    res = _run_device(in_maps)
    stats = np.stack([np.asarray(res.results[i]["stats"]) for i in range(N_CORES)])
    stats = stats.reshape(B_FULL, C, NCHUNK * 8)
    return _assemble(stats)


# revision 7
# speedup vs baseline: 1.3209x; 1.0280x over previous
"""Trainium2 Bass kernel for nn_NetStackedHourglass_2 keypoint reduction.

Full inputs in, full output out.  Pure data-parallel across 8 NeuronCores
(32 batches each).  Host pre-casts the five input streams to bf16 and
interleaves them into ONE DRAM tensor per core laid out
[ROWS, NCHUNK, 7, CHUNK] with planes (m, fd, bd, fvx, fvy, bvx, bvy): the
kernel is HBM-bandwidth bound, so bf16 halves device traffic (quantization
error ~2.5e-4 rel, ~80x inside the 2e-2 gate) and doubles DVE throughput
via the 2x packed mode.  Each (group, half) needs a single 3.67 MB DMA
(28 KB contiguous per partition), alternated across both HWDGE rings.
All compute slices are contiguous; DVE ops write in place over dead
planes; accumulators stay f32; msum runs on ScalarE with an SBUF scratch
output.  The tiny [B,20,*] -> [B,21,2] keypoint assembly runs on host.
"""

import sys

if "/opt/trn_rl_repo" not in sys.path:
    sys.path.insert(0, "/opt/trn_rl_repo")

import numpy as np

import concourse.bass as bass
import concourse.tile as tile
from concourse import mybir
from concourse.bass_utils import run_bass_kernel_spmd

N_CORES = 8
B_FULL = 256
B_SHARD = B_FULL // N_CORES  # 32
C = 20
RES = 64
SPATIAL = RES * RES          # 4096
ROWS = B_SHARD * C           # 640 (b,c) rows per core
P = 128                      # partitions
GROUPS = ROWS // P           # 5
CHUNK = 4096                 # spatial elements per tile (full spatial row)
NCHUNK = SPATIAL // CHUNK    # 1
EPS = 1e-6

F32 = mybir.dt.float32
BF16 = mybir.dt.bfloat16


def _build_program(repeat: int = 1) -> bass.Bass:
    nc = bass.Bass()

    comb = nc.declare_dram_parameter(
        "comb", [ROWS, NCHUNK, 7, CHUNK], BF16, isOutput=False
    )
    # rows: locx for chunk 0, then locy (locx for chunk ci = locx0 + 32*ci,
    # fixed up on host via msum)
    loc = nc.declare_dram_parameter("loc_const", [2, CHUNK], BF16, isOutput=False)
    # raw per-chunk accumulators: 0:fx 1:fy 2:bx 3:by 4:lx 5:ly 6:msum 7:pad
    stats = nc.declare_dram_parameter("stats", [ROWS, NCHUNK * 8], F32, isOutput=True)

    MULT = mybir.AluOpType.mult

    with tile.TileContext(nc) as tc:
        with (
            tc.tile_pool(name="singles", bufs=1) as singles,
            tc.tile_pool(name="io", bufs=3) as io,
            tc.tile_pool(name="scr", bufs=2) as scr,
            tc.tile_pool(name="acc", bufs=4) as accp,
        ):
            locx = singles.tile([P, CHUNK], BF16, tag="locx")
            nc.gpsimd.dma_start(out=locx, in_=loc[0:1, :].to_broadcast([P, CHUNK]))
            locy = singles.tile([P, CHUNK], BF16, tag="locy")
            nc.gpsimd.dma_start(out=locy, in_=loc[1:2, :].to_broadcast([P, CHUNK]))

            rings = [nc.sync, nc.scalar]
            i = 0
            for g in range(GROUPS * repeat):
                g = g % GROUPS
                r0 = g * P
                acc_t = accp.tile([P, NCHUNK, 8], F32, tag="acc")
                nc.vector.memset(acc_t[:, :, 7:8], 0.0)
                for ci in range(NCHUNK):
                    t = io.tile([P, 7, CHUNK], BF16, tag="comb")
                    rings[i % 2].dma_start(out=t, in_=comb[r0 : r0 + P, ci, :, :])
                    i += 1

                    # msum on ScalarE (activation w/ accumulate; out -> SBUF
                    # scratch)
                    mscr = scr.tile([P, CHUNK], BF16, tag="mscr")
                    nc.scalar.activation(
                        out=mscr,
                        in_=t[:, 0],
                        func=mybir.ActivationFunctionType.Copy,
                        accum_out=acc_t[:, ci, 6:7],
                    )

                    # t_f = m*fd over fd's plane; t_b = m*bd over bd's plane
                    nc.vector.tensor_mul(t[:, 1], t[:, 0], t[:, 1])
                    nc.vector.tensor_mul(t[:, 2], t[:, 0], t[:, 2])

                    def fused_dot(q_in, q_t, qi, scale):
                        # t[:,q_in] = (t[:,q_in] * scale) * q_t ; accum = sum
                        nc.vector.scalar_tensor_tensor(
                            out=t[:, q_in],
                            in0=t[:, q_in],
                            scalar=scale,
                            in1=q_t,
                            op0=MULT,
                            op1=MULT,
                            accum_out=acc_t[:, ci, qi : qi + 1],
                        )

                    fused_dot(3, t[:, 1], 0, 64.0)   # fvx * t_f
                    fused_dot(4, t[:, 1], 1, 64.0)   # fvy * t_f (t_f dead)
                    fused_dot(5, t[:, 2], 2, 64.0)   # bvx * t_b
                    # bvy vote: plane 6 is host-prescaled by 64, so this is a
                    # packed TT product; its sum joins the ScalarE passes
                    nc.vector.tensor_mul(t[:, 6], t[:, 2], t[:, 6])
                    # mask-location moments: packed TT products over dead
                    # planes 1/2, sums on ScalarE (which has slack)
                    nc.vector.tensor_mul(t[:, 1], t[:, 0], locx)
                    nc.vector.tensor_mul(t[:, 2], t[:, 0], locy)
                    for qi, plane in ((4, 1), (5, 2), (3, 6)):
                        lscr = scr.tile([P, CHUNK], BF16, tag="mscr")
                        nc.scalar.activation(
                            out=lscr,
                            in_=t[:, plane],
                            func=mybir.ActivationFunctionType.Copy,
                            accum_out=acc_t[:, ci, qi : qi + 1],
                        )

                nc.sync.dma_start(
                    out=stats[r0 : r0 + P, :],
                    in_=acc_t.rearrange("p a b -> p (a b)"),
                )

    from concourse.library_overlay import lower_extended_insts

    lower_extended_insts(nc)
    _legalize_waits(nc)
    return nc


def _legalize_waits(nc) -> None:
    """walrus codegen allows 1 sync-wait per instruction (2 for
    EventSemaphore). Hoist excess waits onto EventSemaphore carriers
    inserted just before the offending instruction on the same engine."""
    for f in nc.m.functions:
        for blk in f.blocks:
            insts = blk.instructions
            new_list = []
            changed = False
            for ins in insts:
                si = getattr(ins, "sync_info", None)
                ow = list(si.on_wait) if (si is not None and si.on_wait) else []
                cap = 2 if isinstance(ins, mybir.InstEventSemaphore) else 1
                if len(ow) > cap:
                    excess, keep = ow[:-cap], ow[-cap:]
                    for j in range(0, len(excess), 2):
                        ev = mybir.InstEventSemaphore(
                            name=f"{ins.name}-lw{j}", ins=[], outs=[]
                        )
                        ev.engine = ins.engine
                        ev.sync_info = mybir.SyncInfo(
                            on_wait=excess[j : j + 2], on_update=[]
                        )
                        new_list.append(ev)
                    ins.sync_info = mybir.SyncInfo(
                        on_wait=keep,
                        on_update=list(si.on_update) if si.on_update else [],
                    )
                    changed = True
                new_list.append(ins)
            if changed:
                blk.instructions.clear()
                blk.instructions.extend(new_list)


_PROGRAM_CACHE: dict = {}


def _get_program() -> bass.Bass:
    if "nc" not in _PROGRAM_CACHE:
        _PROGRAM_CACHE["nc"] = _build_program()
    return _PROGRAM_CACHE["nc"]


def _run_device(in_maps, trace=False, **kwargs):
    nc = _get_program()
    return run_bass_kernel_spmd(nc, in_maps, list(range(N_CORES)), trace=trace, **kwargs)


def _make_in_maps(front_vec, front_dis, back_vec, back_dis, ske_mask):
    import ml_dtypes

    bf = ml_dtypes.bfloat16
    fv = np.asarray(front_vec, dtype=np.float32)
    fd = np.asarray(front_dis, dtype=np.float32)
    bv = np.asarray(back_vec, dtype=np.float32)
    bd = np.asarray(back_dis, dtype=np.float32)
    m = np.asarray(ske_mask, dtype=np.float32)
    B = m.shape[0]
    R = B * C  # 5120 rows total

    comb = np.empty((R, NCHUNK, 7, CHUNK), bf)

    def rows(x):  # [R, SPATIAL] -> [R, NCHUNK, CHUNK]
        return x.reshape(R, NCHUNK, CHUNK)

    comb[:, :, 0] = rows(m.reshape(R, SPATIAL)).astype(bf)
    comb[:, :, 1] = rows(fd.reshape(R, SPATIAL)).astype(bf)
    comb[:, :, 2] = rows(bd.reshape(R, SPATIAL)).astype(bf)
    fvp = fv.reshape(R, SPATIAL, 2)
    bvp = bv.reshape(R, SPATIAL, 2)
    comb[:, :, 3] = fvp[:, :, 0].reshape(R, NCHUNK, CHUNK).astype(bf)
    comb[:, :, 4] = fvp[:, :, 1].reshape(R, NCHUNK, CHUNK).astype(bf)
    comb[:, :, 5] = bvp[:, :, 0].reshape(R, NCHUNK, CHUNK).astype(bf)
    comb[:, :, 6] = (bvp[:, :, 1] * np.float32(RES)).reshape(R, NCHUNK, CHUNK).astype(bf)

    p = np.arange(CHUNK)
    loc_const = np.ascontiguousarray(
        np.stack([(p // RES).astype(bf), (p % RES).astype(bf)])
    )

    in_maps = []
    for i in range(N_CORES):
        sl = slice(i * ROWS, (i + 1) * ROWS)
        in_maps.append({"comb": comb[sl], "loc_const": loc_const})
    return in_maps


def _assemble(stats: np.ndarray) -> np.ndarray:
    """stats: [B, 20, NCHUNK*8] raw accumulators -> kp [B, 21, 2]."""
    B = stats.shape[0]
    acc = stats.reshape(B, C, NCHUNK, 8).astype(np.float32)
    s = acc.sum(axis=2)
    for ci in range(1, NCHUNK):
        s[:, :, 4] += np.float32(ci * (CHUNK // RES)) * acc[:, :, ci, 6]
    msum = s[:, :, 6]
    r = np.float32(1.0) / (msum + np.float32(EPS))
    F_ = np.stack([(s[:, :, 0] + s[:, :, 4]) * r, (s[:, :, 1] + s[:, :, 5]) * r], -1)
    Bk = np.stack([(s[:, :, 2] + s[:, :, 4]) * r, (s[:, :, 3] + s[:, :, 5]) * r], -1)

    root_terms = np.where(
        (msum[:, ::4] != 0.0)[..., None], Bk[:, ::4], np.float32(0.0)
    )  # [B,5,2]
    kp0 = root_terms.sum(axis=1, dtype=np.float32) / np.float32(5.0)  # [B,2]

    Fg = F_.reshape(B, 5, 4, 2)
    Bg = Bk.reshape(B, 5, 4, 2)
    tail = np.stack(
        [
            Fg[:, :, 3],
            (Fg[:, :, 2] + Bg[:, :, 3]) * np.float32(0.5),
            (Fg[:, :, 1] + Bg[:, :, 2]) * np.float32(0.5),
            (Fg[:, :, 0] + Bg[:, :, 1]) * np.float32(0.5),
        ],
        axis=2,
    )  # [B,5,4,2]
    kp = np.concatenate([kp0[:, None], tail.reshape(B, 20, 2)], axis=1)
    return (kp * np.float32(4.0)).astype(np.float32)


def kernel(front_vec, front_dis, back_vec, back_dis, ske_mask) -> np.ndarray:
    in_maps = _make_in_maps(front_vec, front_dis, back_vec, back_dis, ske_mask)
    res = _run_device(in_maps)
    stats = np.stack([np.asarray(res.results[i]["stats"]) for i in range(N_CORES)])
    stats = stats.reshape(B_FULL, C, NCHUNK * 8)
    return _assemble(stats)
